# revision 9
# baseline (speedup 1.0000x reference)
"""Binary linear layer (sign(x) @ sign(w)) on 8 trn2 NeuronCores.

Strategy
--------
Data-parallel: x is split into 8 row-blocks of 1024; the 4096x4096 weight is
replicated. Each core computes out_shard = sign(x_shard) @ sign(w).

All products are +/-1 and row sums are integers <= 4096, so the matmul is
EXACT in bf16 with fp32 PSUM accumulation. On device we binarize to +/-0.5
(one tensor_scalar op: (v >= 0) - 0.5), matmul in bf16 (products +/-0.25,
exact), and multiply the PSUM->SBUF copy by 4 (exact power of two). The
result is bit-identical to the fp32 reference.

The host pre-transposes each x shard to [d_in, n_per] (the PE contraction dim
must live on SBUF partitions) and pre-casts both inputs to bf16 (sign-
preserving for every representable magnitude in these distributions; halves
HBM traffic). All arithmetic happens on device.
"""

import numpy as np
import ml_dtypes

N_TOTAL, D_IN, D_OUT = 8192, 4096, 4096
N_CORES = 8
N_PER = N_TOTAL // N_CORES

# "bf16": bf16 matmul (exact).  "fp8dr": fp8e4 DoubleRow matmul (exact, 2
# virtual PE rows per cell -> half the matmul instructions).
MODE = "bf16"

_PROGRAM_CACHE = {}


def build_program(n_per=N_PER, d_in=D_IN, d_out=D_OUT, num_devices=N_CORES,
                  mode=None):
    """Build + compile the SPMD Bass program (same program on every core)."""
    import concourse.bass as bass  # noqa: F401
    from concourse import bacc, mybir, tile
    from concourse.bass import ds

    if mode is None:
        mode = MODE
    BF = mybir.dt.bfloat16
    F32 = mybir.dt.float32
    FP8 = mybir.dt.float8e4
    MMDT = FP8 if mode == "fp8dr" else BF  # matmul operand dtype
    P = 128
    NW = 512  # n-chunk width = one PSUM bank of fp32
    KT = d_in // P      # k-tiles
    MT = n_per // P     # m-tiles per core
    NCH = d_out // NW   # n-chunks
    ge = mybir.AluOpType.is_ge
    sub = mybir.AluOpType.subtract
    Copy = mybir.ActivationFunctionType.Copy
    perf_mode = mybir.MatmulPerfMode.DoubleRow if mode == "fp8dr" else None

    nc = bacc.Bacc(
        "TRN2",
        target_bir_lowering=False,
        debug=False,
        enable_asserts=False,
        num_devices=num_devices,
    )
    xt = nc.declare_dram_parameter("xt", [d_in, n_per], BF, isOutput=False)
    w = nc.declare_dram_parameter("w", [d_in, d_out], BF, isOutput=False)
    out = nc.declare_dram_parameter("out", [n_per, d_out], F32, isOutput=True)

    # HBM-side access patterns with the k-tile index folded into partitions.
    xt_r = xt.ap().rearrange("(kt p) m -> p kt m", p=P)        # [128, KT, n_per]
    w_r = w.ap().rearrange("(kt p) n -> p kt n", p=P)          # [128, KT, d_out]

    fp8 = mode == "fp8dr"
    if fp8:
        assert KT % 2 == 0

    with tile.TileContext(nc) as tc:
        with (
            tc.tile_pool(name="xpool", bufs=1) as xpool,
            tc.tile_pool(name="spool", bufs=2) as spool,
            tc.tile_pool(name="wpool", bufs=2) as wpool,
            tc.tile_pool(name="opool", bufs=4) as opool,
            tc.tile_pool(name="psum", bufs=4, space="PSUM") as pspool,
        ):
            # ---- load + binarize x (resident for the whole kernel) ----
            xb = xpool.tile([P, KT * n_per], MMDT, tag="xb")
            xb3 = xb[:, :].rearrange("p (kt m) -> p kt m", kt=KT)
            X_CH = min(8, KT)
            kt_per = KT // X_CH
            for c in range(X_CH):
                ktsl = ds(c * kt_per, kt_per)
                fsl = ds(c * kt_per * n_per, kt_per * n_per)
                if fp8:
                    xs = spool.tile([P, kt_per * n_per], BF, tag="xs")
                    xs3 = xs[:, :].rearrange("p (kt m) -> p kt m", kt=kt_per)
                    nc.sync.dma_start(out=xs3[:, :, :], in_=xt_r[:, ktsl, :])
                    nc.vector.tensor_scalar(
                        xb[:, fsl], xs[:, :], 0.0, 0.5, ge, sub
                    )
                else:
                    nc.sync.dma_start(out=xb3[:, ktsl, :], in_=xt_r[:, ktsl, :])
                    nc.vector.tensor_scalar(
                        xb[:, fsl], xb[:, fsl], 0.0, 0.5, ge, sub
                    )

            # ---- stream w by n-chunk; binarize; matmul ----
            for nt in range(NCH):
                nsl = ds(nt * NW, NW)
                if fp8:
                    ws = spool.tile([P, KT * NW], BF, tag="ws")
                    ws3 = ws[:, :].rearrange("p (kt n) -> p kt n", kt=KT)
                    nc.sync.dma_start(out=ws3[:, :, :], in_=w_r[:, :, nsl])
                    src = ws
                else:
                    ws = wpool.tile([P, KT * NW], BF, tag="wb")
                    ws3 = ws[:, :].rearrange("p (kt n) -> p kt n", kt=KT)
                    nc.sync.dma_start(out=ws3[:, :, :], in_=w_r[:, :, nsl])
                    src = ws
                if fp8:
                    wb = wpool.tile([P, KT * NW], FP8, tag="wb")
                else:
                    wb = ws
                wb3 = wb[:, :].rearrange("p (kt n) -> p kt n", kt=KT)
                BIN_W = min(4 * NW, KT * NW)  # 4 k-tiles per DVE op
                for c in range(KT * NW // BIN_W):
                    sl = ds(c * BIN_W, BIN_W)
                    nc.vector.tensor_scalar(
                        wb[:, sl], src[:, sl], 0.0, 0.5, ge, sub
                    )

                for mt in range(MT):
                    ps = pspool.tile([P, NW], F32, tag="ps")
                    if fp8:
                        for t in range(KT // 2):
                            nc.tensor.matmul(
                                ps[:, :],
                                lhsT=xb3[:, 2 * t : 2 * t + 2,
                                         ds(mt * P, P)],
                                rhs=wb3[:, 2 * t : 2 * t + 2, :],
                                start=(t == 0),
                                stop=(t == KT // 2 - 1),
                                perf_mode=perf_mode,
                            )
                    else:
                        for kt in range(KT):
                            nc.tensor.matmul(
                                ps[:, :],
                                lhsT=xb[:, ds(kt * n_per + mt * P, P)],
                                rhs=wb[:, ds(kt * NW, NW)],
                                start=(kt == 0),
                                stop=(kt == KT - 1),
                            )
                    ot = opool.tile([P, NW], F32, tag="ot")
                    nc.scalar.activation(ot[:, :], ps[:, :], Copy, 0.0, 4.0)
                    nc.sync.dma_start(
                        out=out[ds(mt * P, P), nsl], in_=ot[:, :]
                    )

    nc.compile()
    return nc


def _get_program():
    key = (N_PER, D_IN, D_OUT, MODE)
    if key not in _PROGRAM_CACHE:
        _PROGRAM_CACHE[key] = build_program()
    return _PROGRAM_CACHE[key]


def shard_inputs(x, weight):
    """Host-side sharding/layout: bf16 cast + per-shard transpose."""
    bf16 = ml_dtypes.bfloat16
    x16 = x.astype(bf16)
    w16 = np.ascontiguousarray(weight.astype(bf16))
    shards = [
        np.ascontiguousarray(x16[i * N_PER : (i + 1) * N_PER].T)
        for i in range(N_CORES)
    ]
    return [{"xt": shards[i], "w": w16} for i in range(N_CORES)]


def kernel(x, weight):
    from concourse.bass_utils import run_bass_kernel_spmd

    nc = _get_program()
    in_maps = shard_inputs(np.asarray(x), np.asarray(weight))
    res = run_bass_kernel_spmd(nc, in_maps, list(range(N_CORES)))
    return np.concatenate(
        [res.results[i]["out"] for i in range(N_CORES)], axis=0
    )


# revision 10
# speedup vs baseline: 1.7266x; 1.7266x over previous
"""Binary linear layer (sign(x) @ sign(w)) on 8 trn2 NeuronCores.

Strategy
--------
Data-parallel: x is split into 8 row-blocks of 1024; the 4096x4096 weight is
replicated. Each core computes out_shard = sign(x_shard) @ sign(w).

All products are +/-1 and row sums are integers <= 4096, so the matmul is
EXACT in bf16 with fp32 PSUM accumulation. On device we binarize to +/-0.5
(one tensor_scalar op: (v >= 0) - 0.5), matmul in bf16 (products +/-0.25,
exact), and multiply the PSUM->SBUF copy by 4 (exact power of two). The
result is bit-identical to the fp32 reference.

The host pre-transposes each x shard to [d_in, n_per] (the PE contraction dim
must live on SBUF partitions) and pre-casts both inputs to bf16 (sign-
preserving for every representable magnitude in these distributions; halves
HBM traffic). All arithmetic happens on device.
"""

import numpy as np
import ml_dtypes

N_TOTAL, D_IN, D_OUT = 8192, 4096, 4096
N_CORES = 8
N_PER = N_TOTAL // N_CORES

# "bf16": bf16 matmul (exact).  "fp8dr": fp8e4 DoubleRow matmul (exact, 2
# virtual PE rows per cell -> half the matmul instructions).
MODE = "fp8dr"

_PROGRAM_CACHE = {}


def build_program(n_per=N_PER, d_in=D_IN, d_out=D_OUT, num_devices=N_CORES,
                  mode=None):
    """Build + compile the SPMD Bass program (same program on every core)."""
    import concourse.bass as bass  # noqa: F401
    from concourse import bacc, mybir, tile
    from concourse.bass import ds

    if mode is None:
        mode = MODE
    BF = mybir.dt.bfloat16
    F32 = mybir.dt.float32
    FP8 = mybir.dt.float8e4
    MMDT = FP8 if mode == "fp8dr" else BF  # matmul operand dtype
    P = 128
    NW = 512  # n-chunk width = one PSUM bank of fp32
    KT = d_in // P      # k-tiles
    MT = n_per // P     # m-tiles per core
    NCH = d_out // NW   # n-chunks
    ge = mybir.AluOpType.is_ge
    sub = mybir.AluOpType.subtract
    Copy = mybir.ActivationFunctionType.Copy
    perf_mode = mybir.MatmulPerfMode.DoubleRow if mode == "fp8dr" else None

    nc = bacc.Bacc(
        "TRN2",
        target_bir_lowering=False,
        debug=False,
        enable_asserts=False,
        num_devices=num_devices,
    )
    xt = nc.declare_dram_parameter("xt", [d_in, n_per], BF, isOutput=False)
    w = nc.declare_dram_parameter("w", [d_in, d_out], BF, isOutput=False)
    out = nc.declare_dram_parameter("out", [n_per, d_out], F32, isOutput=True)

    # HBM-side access patterns with the k-tile index folded into partitions.
    xt_r = xt.ap().rearrange("(kt p) m -> p kt m", p=P)        # [128, KT, n_per]
    w_r = w.ap().rearrange("(kt p) n -> p kt n", p=P)          # [128, KT, d_out]

    fp8 = mode == "fp8dr"
    if fp8:
        assert KT % 2 == 0

    with tile.TileContext(nc) as tc:
        with (
            tc.tile_pool(name="xpool", bufs=1) as xpool,
            tc.tile_pool(name="spool", bufs=2) as spool,
            tc.tile_pool(name="wpool", bufs=2) as wpool,
            tc.tile_pool(name="opool", bufs=4) as opool,
            tc.tile_pool(name="psum", bufs=4, space="PSUM") as pspool,
        ):
            # ---- load + binarize x (resident for the whole kernel) ----
            xb = xpool.tile([P, KT * n_per], MMDT, tag="xb")
            xb3 = xb[:, :].rearrange("p (kt m) -> p kt m", kt=KT)
            X_CH = min(8, KT)
            kt_per = KT // X_CH
            for c in range(X_CH):
                ktsl = ds(c * kt_per, kt_per)
                fsl = ds(c * kt_per * n_per, kt_per * n_per)
                if fp8:
                    xs = spool.tile([P, kt_per * n_per], BF, tag="xs")
                    xs3 = xs[:, :].rearrange("p (kt m) -> p kt m", kt=kt_per)
                    nc.sync.dma_start(out=xs3[:, :, :], in_=xt_r[:, ktsl, :])
                    nc.vector.tensor_scalar(
                        xb[:, fsl], xs[:, :], 0.0, 0.5, ge, sub
                    )
                else:
                    nc.sync.dma_start(out=xb3[:, ktsl, :], in_=xt_r[:, ktsl, :])
                    nc.vector.tensor_scalar(
                        xb[:, fsl], xb[:, fsl], 0.0, 0.5, ge, sub
                    )

            # ---- stream w by n-chunk; binarize; matmul ----
            for nt in range(NCH):
                nsl = ds(nt * NW, NW)
                if fp8:
                    ws = spool.tile([P, KT * NW], BF, tag="ws")
                    ws3 = ws[:, :].rearrange("p (kt n) -> p kt n", kt=KT)
                    nc.sync.dma_start(out=ws3[:, :, :], in_=w_r[:, :, nsl])
                    src = ws
                else:
                    ws = wpool.tile([P, KT * NW], BF, tag="wb")
                    ws3 = ws[:, :].rearrange("p (kt n) -> p kt n", kt=KT)
                    nc.sync.dma_start(out=ws3[:, :, :], in_=w_r[:, :, nsl])
                    src = ws
                if fp8:
                    wb = wpool.tile([P, KT * NW], FP8, tag="wb")
                else:
                    wb = ws
                wb3 = wb[:, :].rearrange("p (kt n) -> p kt n", kt=KT)
                BIN_W = min(4 * NW, KT * NW)  # 4 k-tiles per DVE op
                for c in range(KT * NW // BIN_W):
                    sl = ds(c * BIN_W, BIN_W)
                    nc.vector.tensor_scalar(
                        wb[:, sl], src[:, sl], 0.0, 0.5, ge, sub
                    )

                for mt in range(MT):
                    ps = pspool.tile([P, NW], F32, tag="ps")
                    if fp8:
                        for t in range(KT // 2):
                            nc.tensor.matmul(
                                ps[:, :],
                                lhsT=xb3[:, 2 * t : 2 * t + 2,
                                         ds(mt * P, P)],
                                rhs=wb3[:, 2 * t : 2 * t + 2, :],
                                start=(t == 0),
                                stop=(t == KT // 2 - 1),
                                perf_mode=perf_mode,
                            )
                    else:
                        for kt in range(KT):
                            nc.tensor.matmul(
                                ps[:, :],
                                lhsT=xb[:, ds(kt * n_per + mt * P, P)],
                                rhs=wb[:, ds(kt * NW, NW)],
                                start=(kt == 0),
                                stop=(kt == KT - 1),
                            )
                    ot = opool.tile([P, NW], F32, tag="ot")
                    nc.scalar.activation(ot[:, :], ps[:, :], Copy, 0.0, 4.0)
                    nc.sync.dma_start(
                        out=out[ds(mt * P, P), nsl], in_=ot[:, :]
                    )

    nc.compile()
    return nc


def _get_program():
    key = (N_PER, D_IN, D_OUT, MODE)
    if key not in _PROGRAM_CACHE:
        _PROGRAM_CACHE[key] = build_program()
    return _PROGRAM_CACHE[key]


def shard_inputs(x, weight):
    """Host-side sharding/layout: bf16 cast + per-shard transpose."""
    bf16 = ml_dtypes.bfloat16
    x16 = x.astype(bf16)
    w16 = np.ascontiguousarray(weight.astype(bf16))
    shards = [
        np.ascontiguousarray(x16[i * N_PER : (i + 1) * N_PER].T)
        for i in range(N_CORES)
    ]
    return [{"xt": shards[i], "w": w16} for i in range(N_CORES)]


def kernel(x, weight):
    from concourse.bass_utils import run_bass_kernel_spmd

    nc = _get_program()
    in_maps = shard_inputs(np.asarray(x), np.asarray(weight))
    res = run_bass_kernel_spmd(nc, in_maps, list(range(N_CORES)))
    return np.concatenate(
        [res.results[i]["out"] for i in range(N_CORES)], axis=0
    )


# revision 16
# speedup vs baseline: 1.8098x; 1.0482x over previous
"""Binary linear layer (sign(x) @ sign(w)) on 8 trn2 NeuronCores.

Strategy
--------
Data-parallel: x is split into 8 row-blocks of 1024; the 4096x4096 weight is
replicated. Each core computes out_shard = sign(x_shard) @ sign(w).

All products are +/-1 and row sums are integers <= 4096, so the matmul is
EXACT in bf16 with fp32 PSUM accumulation. On device we binarize to +/-0.5
(one tensor_scalar op: (v >= 0) - 0.5), matmul in bf16 (products +/-0.25,
exact), and multiply the PSUM->SBUF copy by 4 (exact power of two). The
result is bit-identical to the fp32 reference.

The host pre-transposes each x shard to [d_in, n_per] (the PE contraction dim
must live on SBUF partitions) and pre-casts both inputs to bf16 (sign-
preserving for every representable magnitude in these distributions; halves
HBM traffic). All arithmetic happens on device.
"""

import numpy as np
import ml_dtypes

N_TOTAL, D_IN, D_OUT = 8192, 4096, 4096
N_CORES = 8
N_PER = N_TOTAL // N_CORES

# "bf16": bf16 matmul (exact).  "fp8dr": fp8e4 DoubleRow matmul (exact, 2
# virtual PE rows per cell -> half the matmul instructions).
MODE = "fp8dr"

_PROGRAM_CACHE = {}


def build_program(n_per=N_PER, d_in=D_IN, d_out=D_OUT, num_devices=N_CORES,
                  mode=None):
    """Build + compile the SPMD Bass program (same program on every core)."""
    import concourse.bass as bass  # noqa: F401
    from concourse import bacc, mybir, tile
    from concourse.bass import ds

    if mode is None:
        mode = MODE
    BF = mybir.dt.bfloat16
    F32 = mybir.dt.float32
    FP8 = mybir.dt.float8e4
    MMDT = FP8 if mode == "fp8dr" else BF  # matmul operand dtype
    P = 128
    NW = 512  # n-chunk width = one PSUM bank of fp32
    KT = d_in // P      # k-tiles
    MT = n_per // P     # m-tiles per core
    NCH = d_out // NW   # n-chunks
    ge = mybir.AluOpType.is_ge
    sub = mybir.AluOpType.subtract
    Copy = mybir.ActivationFunctionType.Copy
    Sign = mybir.ActivationFunctionType.Sign
    perf_mode = mybir.MatmulPerfMode.DoubleRow if mode == "fp8dr" else None
    # fp8 mode: x -> +/-1 via ACT Sign (host patches exact zeros to +tiny),
    # w -> +/-0.5 via DVE is_ge (zero-safe); products +/-0.5, copy scale 2.
    # bf16 mode: both +/-0.5 on DVE; copy scale 4.
    OUT_SCALE = 2.0 if mode == "fp8dr" else 4.0

    nc = bacc.Bacc(
        "TRN2",
        target_bir_lowering=False,
        debug=False,
        enable_asserts=False,
        num_devices=num_devices,
    )
    xt = nc.declare_dram_parameter("xt", [d_in, n_per], BF, isOutput=False)
    w = nc.declare_dram_parameter("w", [d_in, d_out], BF, isOutput=False)
    out = nc.declare_dram_parameter("out", [n_per, d_out], F32, isOutput=True)

    # HBM-side access patterns with the k-tile index folded into partitions.
    xt_r = xt.ap().rearrange("(kt p) m -> p kt m", p=P)        # [128, KT, n_per]
    w_r = w.ap().rearrange("(kt p) n -> p kt n", p=P)          # [128, KT, d_out]

    fp8 = mode == "fp8dr"
    if fp8:
        assert KT % 2 == 0

    with tile.TileContext(nc) as tc:
        with (
            tc.tile_pool(name="xpool", bufs=1) as xpool,
            tc.tile_pool(name="spool", bufs=2) as spool,
            tc.tile_pool(name="wpool", bufs=3 if fp8 else 2) as wpool,
            tc.tile_pool(name="opool", bufs=6) as opool,
            tc.tile_pool(name="psum", bufs=6, space="PSUM") as pspool,
        ):
            # ---- load + binarize x (resident for the whole kernel) ----
            xb = xpool.tile([P, KT * n_per], MMDT, tag="xb")
            xb3 = xb[:, :].rearrange("p (kt m) -> p kt m", kt=KT)
            X_CH = min(8, KT)
            kt_per = KT // X_CH
            for c in range(X_CH):
                ktsl = ds(c * kt_per, kt_per)
                fsl = ds(c * kt_per * n_per, kt_per * n_per)
                if fp8:
                    xs = spool.tile([P, kt_per * n_per], BF, tag="xs")
                    xs3 = xs[:, :].rearrange("p (kt m) -> p kt m", kt=kt_per)
                    nc.sync.dma_start(out=xs3[:, :, :], in_=xt_r[:, ktsl, :])
                    nc.scalar.activation(xb[:, fsl], xs[:, :], Sign)
                else:
                    nc.sync.dma_start(out=xb3[:, ktsl, :], in_=xt_r[:, ktsl, :])
                    nc.vector.tensor_scalar(
                        xb[:, fsl], xb[:, fsl], 0.0, 0.5, ge, sub
                    )

            # ---- stream w by n-chunk; binarize; matmul ----
            for nt in range(NCH):
                nsl = ds(nt * NW, NW)
                if fp8:
                    ws = spool.tile([P, KT * NW], BF, tag="ws")
                else:
                    ws = wpool.tile([P, KT * NW], BF, tag="wb")
                ws3 = ws[:, :].rearrange("p (kt n) -> p kt n", kt=KT)
                # two half-chunk DMAs so binarize/MMs start on the first half
                half = max(1, KT // 2)
                for h in range(KT // half):
                    hsl = ds(h * half, half)
                    nc.sync.dma_start(
                        out=ws3[:, hsl, :], in_=w_r[:, hsl, nsl]
                    )
                src = ws
                if fp8:
                    wb = wpool.tile([P, KT * NW], FP8, tag="wb")
                else:
                    wb = ws
                wb3 = wb[:, :].rearrange("p (kt n) -> p kt n", kt=KT)
                BIN_W = min(4 * NW, KT * NW)  # 4 k-tiles per DVE op
                for c in range(KT * NW // BIN_W):
                    sl = ds(c * BIN_W, BIN_W)
                    nc.vector.tensor_scalar(
                        wb[:, sl], src[:, sl], 0.0, 0.5, ge, sub
                    )

                for mt in range(MT):
                    ps = pspool.tile([P, NW], F32, tag="ps")
                    if fp8:
                        for t in range(KT // 2):
                            nc.tensor.matmul(
                                ps[:, :],
                                lhsT=xb3[:, 2 * t : 2 * t + 2,
                                         ds(mt * P, P)],
                                rhs=wb3[:, 2 * t : 2 * t + 2, :],
                                start=(t == 0),
                                stop=(t == KT // 2 - 1),
                                perf_mode=perf_mode,
                            )
                    else:
                        for kt in range(KT):
                            nc.tensor.matmul(
                                ps[:, :],
                                lhsT=xb[:, ds(kt * n_per + mt * P, P)],
                                rhs=wb[:, ds(kt * NW, NW)],
                                start=(kt == 0),
                                stop=(kt == KT - 1),
                            )
                    ot = opool.tile([P, NW], F32, tag="ot")
                    nc.scalar.activation(ot[:, :], ps[:, :], Copy, 0.0, OUT_SCALE)
                    nc.sync.dma_start(
                        out=out[ds(mt * P, P), nsl], in_=ot[:, :]
                    )

    nc.compile()
    return nc


def _get_program():
    key = (N_PER, D_IN, D_OUT, MODE)
    if key not in _PROGRAM_CACHE:
        _PROGRAM_CACHE[key] = build_program()
    return _PROGRAM_CACHE[key]


def shard_inputs(x, weight):
    """Host-side sharding/layout: bf16 cast + per-shard transpose."""
    bf16 = ml_dtypes.bfloat16
    x16 = x.astype(bf16)
    if MODE == "fp8dr":
        # x is binarized on-device with ACT Sign, which maps 0 -> 0 while
        # the reference maps 0 -> +1. Patch exact zeros (incl. -0.0) to a
        # tiny positive so Sign agrees with (v >= 0) everywhere.
        zmask = x16 == 0
        if zmask.any():
            x16 = np.where(zmask, np.asarray(1e-20, bf16), x16)
    w16 = np.ascontiguousarray(weight.astype(bf16))
    shards = [
        np.ascontiguousarray(x16[i * N_PER : (i + 1) * N_PER].T)
        for i in range(N_CORES)
    ]
    return [{"xt": shards[i], "w": w16} for i in range(N_CORES)]


def kernel(x, weight):
    from concourse.bass_utils import run_bass_kernel_spmd

    nc = _get_program()
    in_maps = shard_inputs(np.asarray(x), np.asarray(weight))
    res = run_bass_kernel_spmd(nc, in_maps, list(range(N_CORES)))
    return np.concatenate(
        [res.results[i]["out"] for i in range(N_CORES)], axis=0
    )


# revision 22
# speedup vs baseline: 1.8487x; 1.0215x over previous
"""Binary linear layer (sign(x) @ sign(w)) on 8 trn2 NeuronCores.

Strategy
--------
Data-parallel: x is split into 8 row-blocks of 1024; the 4096x4096 weight is
replicated. Each core computes out_shard = sign(x_shard) @ sign(w).

All products are +/-1 and row sums are integers <= 4096, so the matmul is
EXACT in bf16 with fp32 PSUM accumulation. On device we binarize to +/-0.5
(one tensor_scalar op: (v >= 0) - 0.5), matmul in bf16 (products +/-0.25,
exact), and multiply the PSUM->SBUF copy by 4 (exact power of two). The
result is bit-identical to the fp32 reference.

The host pre-transposes each x shard to [d_in, n_per] (the PE contraction dim
must live on SBUF partitions) and pre-casts both inputs to bf16 (sign-
preserving for every representable magnitude in these distributions; halves
HBM traffic). All arithmetic happens on device.
"""

import numpy as np
import ml_dtypes

N_TOTAL, D_IN, D_OUT = 8192, 4096, 4096
N_CORES = 8
N_PER = N_TOTAL // N_CORES

# "bf16": bf16 matmul (exact).  "fp8dr": fp8e4 DoubleRow matmul (exact, 2
# virtual PE rows per cell -> half the matmul instructions).
MODE = "fp8dr"

_PROGRAM_CACHE = {}


def build_program(n_per=N_PER, d_in=D_IN, d_out=D_OUT, num_devices=N_CORES,
                  mode=None):
    """Build + compile the SPMD Bass program (same program on every core)."""
    import concourse.bass as bass  # noqa: F401
    from concourse import bacc, mybir, tile
    from concourse.bass import ds

    if mode is None:
        mode = MODE
    BF = mybir.dt.bfloat16
    F32 = mybir.dt.float32
    FP8 = mybir.dt.float8e4
    MMDT = FP8 if mode == "fp8dr" else BF  # matmul operand dtype
    P = 128
    NW = 512  # n-chunk width = one PSUM bank of fp32
    KT = d_in // P      # k-tiles
    MT = n_per // P     # m-tiles per core
    NCH = d_out // NW   # n-chunks
    ge = mybir.AluOpType.is_ge
    sub = mybir.AluOpType.subtract
    Copy = mybir.ActivationFunctionType.Copy
    Sign = mybir.ActivationFunctionType.Sign
    perf_mode = mybir.MatmulPerfMode.DoubleRow if mode == "fp8dr" else None
    # fp8 mode: x -> +/-1 via ACT Sign (host patches exact zeros to +tiny),
    # w -> +/-0.5 via DVE is_ge (zero-safe); products +/-0.5, copy scale 2.
    # bf16 mode: both +/-0.5 on DVE; copy scale 4.
    OUT_SCALE = 2.0 if mode == "fp8dr" else 4.0

    nc = bacc.Bacc(
        "TRN2",
        target_bir_lowering=False,
        debug=False,
        enable_asserts=False,
        num_devices=num_devices,
    )
    xt = nc.declare_dram_parameter("xt", [d_in, n_per], BF, isOutput=False)
    w = nc.declare_dram_parameter("w", [d_in, d_out], BF, isOutput=False)
    out = nc.declare_dram_parameter("out", [n_per, d_out], F32, isOutput=True)

    # HBM-side access patterns with the k-tile index folded into partitions.
    xt_r = xt.ap().rearrange("(kt p) m -> p kt m", p=P)        # [128, KT, n_per]
    w_r = w.ap().rearrange("(kt p) n -> p kt n", p=P)          # [128, KT, d_out]

    fp8 = mode == "fp8dr"
    if fp8:
        assert KT % 2 == 0

    with tile.TileContext(nc) as tc:
        with (
            tc.tile_pool(name="xpool", bufs=1) as xpool,
            tc.tile_pool(name="spool", bufs=2) as spool,
            tc.tile_pool(name="wpool", bufs=3 if fp8 else 2) as wpool,
            tc.tile_pool(name="opool", bufs=8) as opool,
            tc.tile_pool(name="psum", bufs=8, space="PSUM") as pspool,
        ):
            xb = xpool.tile([P, KT * n_per], MMDT, tag="xb")
            xb3 = xb[:, :].rearrange("p (kt m) -> p kt m", kt=KT)
            X_CH = min(8, KT)
            kt_per = KT // X_CH

            def load_x_chunk(c):
                ktsl = ds(c * kt_per, kt_per)
                fsl = ds(c * kt_per * n_per, kt_per * n_per)
                if fp8:
                    xs = spool.tile([P, kt_per * n_per], BF, tag="xs")
                    xs3 = xs[:, :].rearrange("p (kt m) -> p kt m", kt=kt_per)
                    nc.sync.dma_start(out=xs3[:, :, :], in_=xt_r[:, ktsl, :])
                    nc.scalar.activation(xb[:, fsl], xs[:, :], Sign)
                else:
                    nc.sync.dma_start(out=xb3[:, ktsl, :], in_=xt_r[:, ktsl, :])
                    nc.vector.tensor_scalar(
                        xb[:, fsl], xb[:, fsl], 0.0, 0.5, ge, sub
                    )

            HALF = max(1, KT // 2)
            N_HALVES = KT // HALF
            BIN_KT = min(4, HALF)  # k-tiles per DVE binarize op

            def load_w_chunk(nt, half):
                """DMA + binarize one half (in k) of w n-chunk nt."""
                nsl = ds(nt * NW, NW)
                ws, wb = w_tiles[nt]
                ws3 = ws[:, :].rearrange("p (kt n) -> p kt n", kt=KT)
                hsl = ds(half * HALF, HALF)
                nc.sync.dma_start(out=ws3[:, hsl, :], in_=w_r[:, hsl, nsl])
                for c in range(HALF // BIN_KT):
                    sl = ds((half * HALF + c * BIN_KT) * NW, BIN_KT * NW)
                    nc.vector.tensor_scalar(
                        wb[:, sl], ws[:, sl], 0.0, 0.5, ge, sub
                    )

            def alloc_w_tiles(nt):
                if fp8:
                    ws = spool.tile([P, KT * NW], BF, tag="ws")
                    wb = wpool.tile([P, KT * NW], FP8, tag="wb")
                else:
                    ws = wpool.tile([P, KT * NW], BF, tag="wb")
                    wb = ws
                w_tiles[nt] = (ws, wb)

            def mm(ps, mt, t, wb3, start, stop):
                if fp8:
                    nc.tensor.matmul(
                        ps[:, :],
                        lhsT=xb3[:, 2 * t : 2 * t + 2, ds(mt * P, P)],
                        rhs=wb3[:, 2 * t : 2 * t + 2, :],
                        start=start, stop=stop, perf_mode=perf_mode,
                    )
                else:
                    nc.tensor.matmul(
                        ps[:, :],
                        lhsT=xb[:, ds(t * n_per + mt * P, P)],
                        rhs=wb3[:, t, :],
                        start=start, stop=stop,
                    )

            def evict(ps, mt, nt):
                ot = opool.tile([P, NW], F32, tag="ot")
                nc.scalar.activation(ot[:, :], ps[:, :], Copy, 0.0, OUT_SCALE)
                nc.sync.dma_start(
                    out=out[ds(mt * P, P), ds(nt * NW, NW)], in_=ot[:, :]
                )

            w_tiles = {}
            NK = KT // 2 if fp8 else KT  # MM k-iterations per psum group

            # Startup interleave: first half of w chunk 0, then x, then the
            # rest of w chunk 0 — so the PE can start at the first x k-tiles
            # and never waits on the second w half.
            alloc_w_tiles(0)
            load_w_chunk(0, 0)
            for c in range(X_CH // 2):
                load_x_chunk(c)
            if N_HALVES > 1:
                load_w_chunk(0, 1)
            for c in range(X_CH // 2, X_CH):
                load_x_chunk(c)

            # n-chunk 0: kt-outer across all MT psum banks, pacing the PE
            # behind the streaming x DMA instead of stalling on full x.
            ps0 = [
                pspool.tile([P, NW], F32, tag="ps", name=f"ps0_{i}")
                for i in range(MT)
            ]
            wb3_0 = w_tiles[0][1][:, :].rearrange("p (kt n) -> p kt n", kt=KT)
            for t in range(NK):
                for mt in range(MT):
                    mm(ps0[mt], mt, t, wb3_0, start=(t == 0), stop=(t == NK - 1))
            for mt in range(MT):
                evict(ps0[mt], mt, 0)

            # n-chunks 1..: mt-outer (staggered psum eviction)
            for nt in range(1, NCH):
                alloc_w_tiles(nt)
                for h in range(N_HALVES):
                    load_w_chunk(nt, h)
                wb3 = w_tiles[nt][1][:, :].rearrange(
                    "p (kt n) -> p kt n", kt=KT
                )
                for mt in range(MT):
                    ps = pspool.tile([P, NW], F32, tag="ps")
                    for t in range(NK):
                        mm(ps, mt, t, wb3, start=(t == 0), stop=(t == NK - 1))
                    evict(ps, mt, nt)

    nc.compile()
    return nc


def _get_program():
    key = (N_PER, D_IN, D_OUT, MODE)
    if key not in _PROGRAM_CACHE:
        _PROGRAM_CACHE[key] = build_program()
    return _PROGRAM_CACHE[key]


def shard_inputs(x, weight):
    """Host-side sharding/layout: bf16 cast + per-shard transpose."""
    bf16 = ml_dtypes.bfloat16
    x16 = x.astype(bf16)
    if MODE == "fp8dr":
        # x is binarized on-device with ACT Sign, which maps 0 -> 0 while
        # the reference maps 0 -> +1. Patch exact zeros (incl. -0.0) to a
        # tiny positive so Sign agrees with (v >= 0) everywhere.
        zmask = x16 == 0
        if zmask.any():
            x16 = np.where(zmask, np.asarray(1e-20, bf16), x16)
    w16 = np.ascontiguousarray(weight.astype(bf16))
    shards = [
        np.ascontiguousarray(x16[i * N_PER : (i + 1) * N_PER].T)
        for i in range(N_CORES)
    ]
    return [{"xt": shards[i], "w": w16} for i in range(N_CORES)]


def kernel(x, weight):
    from concourse.bass_utils import run_bass_kernel_spmd

    nc = _get_program()
    in_maps = shard_inputs(np.asarray(x), np.asarray(weight))
    res = run_bass_kernel_spmd(nc, in_maps, list(range(N_CORES)))
    return np.concatenate(
        [res.results[i]["out"] for i in range(N_CORES)], axis=0
    )


# revision 31
# speedup vs baseline: 1.9985x; 1.0810x over previous
"""Binary linear layer (sign(x) @ sign(w)) on 8 trn2 NeuronCores.

Strategy
--------
Data-parallel: x is split into 8 row-blocks of 1024; the 4096x4096 weight is
replicated. Each core computes out_shard = sign(x_shard) @ sign(w).

All products are +/-1 and row sums are integers <= 4096, so the matmul is
EXACT in bf16 with fp32 PSUM accumulation. On device we binarize to +/-0.5
(one tensor_scalar op: (v >= 0) - 0.5), matmul in bf16 (products +/-0.25,
exact), and multiply the PSUM->SBUF copy by 4 (exact power of two). The
result is bit-identical to the fp32 reference.

The host pre-transposes each x shard to [d_in, n_per] (the PE contraction dim
must live on SBUF partitions) and pre-casts both inputs to bf16 (sign-
preserving for every representable magnitude in these distributions; halves
HBM traffic). All arithmetic happens on device.
"""

import numpy as np
import ml_dtypes

N_TOTAL, D_IN, D_OUT = 8192, 4096, 4096
N_CORES = 8
N_PER = N_TOTAL // N_CORES


def fp8_in(mode):
    return mode == "fp8dr"

# "bf16": bf16 matmul (exact).  "fp8dr": fp8e4 DoubleRow matmul (exact, 2
# virtual PE rows per cell -> half the matmul instructions).
MODE = "fp8dr"

_PROGRAM_CACHE = {}


def build_program(n_per=N_PER, d_in=D_IN, d_out=D_OUT, num_devices=N_CORES,
                  mode=None):
    """Build + compile the SPMD Bass program (same program on every core)."""
    import concourse.bass as bass  # noqa: F401
    from concourse import bacc, mybir, tile
    from concourse.bass import ds

    if mode is None:
        mode = MODE
    BF = mybir.dt.bfloat16
    F32 = mybir.dt.float32
    FP8 = mybir.dt.float8e4
    MMDT = FP8 if mode == "fp8dr" else BF  # matmul operand dtype
    P = 128
    NW = 512  # n-chunk width = one PSUM bank of fp32
    KT = d_in // P      # k-tiles
    MT = n_per // P     # m-tiles per core
    NCH = d_out // NW   # n-chunks
    ge = mybir.AluOpType.is_ge
    sub = mybir.AluOpType.subtract
    Copy = mybir.ActivationFunctionType.Copy
    Sign = mybir.ActivationFunctionType.Sign
    perf_mode = mybir.MatmulPerfMode.DoubleRow if mode == "fp8dr" else None
    # fp8 mode: x -> +/-1 via ACT Sign (host patches exact zeros to +tiny),
    # w -> +/-0.5 via DVE is_ge (zero-safe); products +/-0.5, copy scale 2.
    # bf16 mode: both +/-0.5 on DVE; copy scale 4.
    OUT_SCALE = 2.0 if mode == "fp8dr" else 4.0

    nc = bacc.Bacc(
        "TRN2",
        target_bir_lowering=False,
        debug=False,
        enable_asserts=False,
        num_devices=num_devices,
    )
    # fp8 mode ships inputs as fp8e4 (host re-encode is sign-exact; see
    # shard_inputs) -> half the HBM traffic of bf16.
    INDT = FP8 if fp8_in(mode) else BF
    xt = nc.declare_dram_parameter("xt", [d_in, n_per], INDT, isOutput=False)
    w = nc.declare_dram_parameter("w", [d_in, d_out], INDT, isOutput=False)
    out = nc.declare_dram_parameter("out", [n_per, d_out], F32, isOutput=True)

    # HBM-side access patterns with the k-tile index folded into partitions.
    xt_r = xt.ap().rearrange("(kt p) m -> p kt m", p=P)        # [128, KT, n_per]
    w_r = w.ap().rearrange("(kt p) n -> p kt n", p=P)          # [128, KT, d_out]

    fp8 = mode == "fp8dr"
    if fp8:
        assert KT % 2 == 0

    with tile.TileContext(nc) as tc:
        with (
            tc.tile_pool(name="xpool", bufs=1) as xpool,
            tc.tile_pool(name="wpool", bufs=4 if fp8 else 2) as wpool,
            tc.tile_pool(name="opool", bufs=8) as opool,
            tc.tile_pool(name="psum", bufs=8, space="PSUM") as pspool,
        ):
            xb = xpool.tile([P, KT * n_per], MMDT, tag="xb")
            xb3 = xb[:, :].rearrange("p (kt m) -> p kt m", kt=KT)
            X_CH = min(8, KT)
            kt_per = KT // X_CH

            def load_x_chunk(c):
                ktsl = ds(c * kt_per, kt_per)
                fsl = ds(c * kt_per * n_per, kt_per * n_per)
                nc.sync.dma_start(out=xb3[:, ktsl, :], in_=xt_r[:, ktsl, :])
                if fp8:
                    # ACT engine; host patched exact zeros so Sign == (v>=0)
                    nc.scalar.activation(xb[:, fsl], xb[:, fsl], Sign)
                else:
                    nc.vector.tensor_scalar(
                        xb[:, fsl], xb[:, fsl], 0.0, 0.5, ge, sub
                    )

            HALF = max(1, KT // 2)
            N_HALVES = KT // HALF
            BIN_KT = min(4, HALF)  # k-tiles per DVE binarize op

            def load_w_chunk(nt, half):
                """DMA + binarize (in place) one k-half of w n-chunk nt."""
                nsl = ds(nt * NW, NW)
                wb = w_tiles[nt]
                wb3 = wb[:, :].rearrange("p (kt n) -> p kt n", kt=KT)
                hsl = ds(half * HALF, HALF)
                nc.sync.dma_start(out=wb3[:, hsl, :], in_=w_r[:, hsl, nsl])
                for c in range(HALF // BIN_KT):
                    sl = ds((half * HALF + c * BIN_KT) * NW, BIN_KT * NW)
                    nc.vector.tensor_scalar(
                        wb[:, sl], wb[:, sl], 0.0, 0.5, ge, sub
                    )

            def alloc_w_tiles(nt):
                wb = wpool.tile([P, KT * NW], MMDT, tag="wb", name=f"wb{nt}")
                w_tiles[nt] = wb

            def mm(ps, mt, t, wb3, start, stop):
                if fp8:
                    nc.tensor.matmul(
                        ps[:, :],
                        lhsT=xb3[:, 2 * t : 2 * t + 2, ds(mt * P, P)],
                        rhs=wb3[:, 2 * t : 2 * t + 2, :],
                        start=start, stop=stop, perf_mode=perf_mode,
                    )
                else:
                    nc.tensor.matmul(
                        ps[:, :],
                        lhsT=xb[:, ds(t * n_per + mt * P, P)],
                        rhs=wb3[:, t, :],
                        start=start, stop=stop,
                    )

            def evict(ps, mt, nt):
                ot = opool.tile([P, NW], F32, tag="ot")
                nc.scalar.activation(ot[:, :], ps[:, :], Copy, 0.0, OUT_SCALE)
                nc.sync.dma_start(
                    out=out[ds(mt * P, P), ds(nt * NW, NW)], in_=ot[:, :]
                )

            w_tiles = {}
            NK = KT // 2 if fp8 else KT  # MM k-iterations per psum group

            # Startup interleave: first half of w chunk 0, then x, then the
            # rest of w chunk 0 — so the PE can start at the first x k-tiles
            # and never waits on the second w half.
            alloc_w_tiles(0)
            load_w_chunk(0, 0)
            for c in range(X_CH // 2):
                load_x_chunk(c)
            if N_HALVES > 1:
                load_w_chunk(0, 1)
            for c in range(X_CH // 2, X_CH):
                load_x_chunk(c)

            # n-chunk 0: kt-outer across all MT psum banks, pacing the PE
            # behind the streaming x DMA instead of stalling on full x.
            ps0 = [
                pspool.tile([P, NW], F32, tag="ps", name=f"ps0_{i}")
                for i in range(MT)
            ]
            wb3_0 = w_tiles[0][:, :].rearrange("p (kt n) -> p kt n", kt=KT)
            for t in range(NK):
                for mt in range(MT):
                    mm(ps0[mt], mt, t, wb3_0, start=(t == 0), stop=(t == NK - 1))
            for mt in range(MT):
                evict(ps0[mt], mt, 0)

            # n-chunks 1..: mt-outer (staggered psum eviction)
            for nt in range(1, NCH):
                alloc_w_tiles(nt)
                for h in range(N_HALVES):
                    load_w_chunk(nt, h)
                wb3 = w_tiles[nt][:, :].rearrange(
                    "p (kt n) -> p kt n", kt=KT
                )
                for mt in range(MT):
                    ps = pspool.tile([P, NW], F32, tag="ps")
                    for t in range(NK):
                        mm(ps, mt, t, wb3, start=(t == 0), stop=(t == NK - 1))
                    evict(ps, mt, nt)

    nc.compile()
    return nc


def _get_program():
    key = (N_PER, D_IN, D_OUT, MODE)
    if key not in _PROGRAM_CACHE:
        _PROGRAM_CACHE[key] = build_program()
    return _PROGRAM_CACHE[key]


def _encode_fp8(v):
    """Sign-exact fp8e4 re-encode of fp32 data for the device binarizer.

    ml_dtypes.float8_e4m3 matches TRN FP8_EXP4 (max 240, overflow saturates
    to +/-Inf, underflow to +/-0 -- sign always survives in the result).
    The only sign-ambiguous encodings are +/-0, which we patch to +/-1:
    +0 covers true zeros (reference maps them to +1) and underflowed
    positives; -0 covers underflowed negatives. After the patch the device
    binarize (v >= 0, or Sign) reproduces sign(original fp32) exactly for
    EVERY possible input value.
    """
    f8 = ml_dtypes.float8_e4m3
    v8 = np.clip(v, -240.0, 240.0).astype(f8)
    z = v8 == 0
    if z.any():
        v8 = np.where(z, np.where(np.signbit(v8), -1.0, 1.0).astype(f8), v8)
    return v8


def shard_inputs(x, weight):
    """Host-side sharding/layout: dtype re-encode + per-shard transpose."""
    if fp8_in(MODE):
        xe = _encode_fp8(x)
        we = _encode_fp8(weight)
    else:
        bf16 = ml_dtypes.bfloat16
        xe = x.astype(bf16)
        we = weight.astype(bf16)
    we = np.ascontiguousarray(we)
    shards = [
        np.ascontiguousarray(xe[i * N_PER : (i + 1) * N_PER].T)
        for i in range(N_CORES)
    ]
    return [{"xt": shards[i], "w": we} for i in range(N_CORES)]


def kernel(x, weight):
    from concourse.bass_utils import run_bass_kernel_spmd

    nc = _get_program()
    in_maps = shard_inputs(np.asarray(x), np.asarray(weight))
    res = run_bass_kernel_spmd(nc, in_maps, list(range(N_CORES)))
    return np.concatenate(
        [res.results[i]["out"] for i in range(N_CORES)], axis=0
    )


# revision 34
# speedup vs baseline: 2.0230x; 1.0123x over previous
"""Binary linear layer (sign(x) @ sign(w)) on 8 trn2 NeuronCores.

Strategy
--------
Data-parallel: x is split into 8 row-blocks of 1024; the 4096x4096 weight is
replicated. Each core computes out_shard = sign(x_shard) @ sign(w).

All products are +/-1 and row sums are integers <= 4096, so the matmul is
exact in low precision with fp32 PSUM accumulation. The fast path ("fp8dr"):

- Host re-encodes both inputs to fp8e4 (sign-exact for every input value --
  see _encode_fp8) and pre-transposes each x shard to [d_in, n_per] so the
  PE contraction dim lands on SBUF partitions. 21 MB HBM in per core.
- Device binarizes x -> +/-1 (ACT engine, Sign) and w -> +/-0.5 (DVE,
  (v>=0)-0.5, in place), then runs fp8 DoubleRow matmuls (2 virtual PE rows
  per cell = 157 TFLOP/s): products +/-0.5, integer-scaled sums, all exact.
- The PSUM->SBUF eviction copy multiplies by 2 (exact power of two).
  Result is bit-identical to the fp32 reference.

n-chunk 0 runs kt-outer across all 8 PSUM banks so the PE paces behind the
streaming x DMA; later chunks run mt-outer with staggered evictions.
Measured ~248 us/kernel (fp8 roofline for the per-core GEMM is ~219 us).
"""

import numpy as np
import ml_dtypes

N_TOTAL, D_IN, D_OUT = 8192, 4096, 4096
N_CORES = 8
N_PER = N_TOTAL // N_CORES


def fp8_in(mode):
    return mode == "fp8dr"

# "bf16": bf16 matmul (exact).  "fp8dr": fp8e4 DoubleRow matmul (exact, 2
# virtual PE rows per cell -> half the matmul instructions).
MODE = "fp8dr"

_PROGRAM_CACHE = {}


def build_program(n_per=N_PER, d_in=D_IN, d_out=D_OUT, num_devices=N_CORES,
                  mode=None):
    """Build + compile the SPMD Bass program (same program on every core)."""
    import concourse.bass as bass  # noqa: F401
    from concourse import bacc, mybir, tile
    from concourse.bass import ds

    if mode is None:
        mode = MODE
    BF = mybir.dt.bfloat16
    F32 = mybir.dt.float32
    FP8 = mybir.dt.float8e4
    MMDT = FP8 if mode == "fp8dr" else BF  # matmul operand dtype
    P = 128
    NW = 512  # n-chunk width = one PSUM bank of fp32
    KT = d_in // P      # k-tiles
    MT = n_per // P     # m-tiles per core
    NCH = d_out // NW   # n-chunks
    ge = mybir.AluOpType.is_ge
    sub = mybir.AluOpType.subtract
    Copy = mybir.ActivationFunctionType.Copy
    Sign = mybir.ActivationFunctionType.Sign
    perf_mode = mybir.MatmulPerfMode.DoubleRow if mode == "fp8dr" else None
    # fp8 mode: x -> +/-1 via ACT Sign (host patches exact zeros to +tiny),
    # w -> +/-0.5 via DVE is_ge (zero-safe); products +/-0.5, copy scale 2.
    # bf16 mode: both +/-0.5 on DVE; copy scale 4.
    OUT_SCALE = 2.0 if mode == "fp8dr" else 4.0

    nc = bacc.Bacc(
        "TRN2",
        target_bir_lowering=False,
        debug=False,
        enable_asserts=False,
        num_devices=num_devices,
    )
    # fp8 mode ships inputs as fp8e4 (host re-encode is sign-exact; see
    # shard_inputs) -> half the HBM traffic of bf16.
    INDT = FP8 if fp8_in(mode) else BF
    xt = nc.declare_dram_parameter("xt", [d_in, n_per], INDT, isOutput=False)
    w = nc.declare_dram_parameter("w", [d_in, d_out], INDT, isOutput=False)
    out = nc.declare_dram_parameter("out", [n_per, d_out], F32, isOutput=True)

    # HBM-side access patterns with the k-tile index folded into partitions.
    xt_r = xt.ap().rearrange("(kt p) m -> p kt m", p=P)        # [128, KT, n_per]
    w_r = w.ap().rearrange("(kt p) n -> p kt n", p=P)          # [128, KT, d_out]

    fp8 = mode == "fp8dr"
    if fp8:
        assert KT % 2 == 0

    with tile.TileContext(nc) as tc:
        with (
            tc.tile_pool(name="xpool", bufs=1) as xpool,
            tc.tile_pool(name="wpool", bufs=4 if fp8 else 2) as wpool,
            tc.tile_pool(name="opool", bufs=8) as opool,
            tc.tile_pool(name="psum", bufs=8, space="PSUM") as pspool,
        ):
            xb = xpool.tile([P, KT * n_per], MMDT, tag="xb")
            xb3 = xb[:, :].rearrange("p (kt m) -> p kt m", kt=KT)
            X_CH = min(16, KT)
            kt_per = KT // X_CH

            def load_x_chunk(c):
                ktsl = ds(c * kt_per, kt_per)
                fsl = ds(c * kt_per * n_per, kt_per * n_per)
                nc.sync.dma_start(out=xb3[:, ktsl, :], in_=xt_r[:, ktsl, :])
                if fp8:
                    # ACT engine; host patched exact zeros so Sign == (v>=0)
                    nc.scalar.activation(xb[:, fsl], xb[:, fsl], Sign)
                else:
                    nc.vector.tensor_scalar(
                        xb[:, fsl], xb[:, fsl], 0.0, 0.5, ge, sub
                    )

            HALF = max(1, KT // 2)
            N_HALVES = KT // HALF
            BIN_KT = min(4, HALF)  # k-tiles per DVE binarize op

            def load_w_chunk(nt, half):
                """DMA + binarize (in place) one k-half of w n-chunk nt."""
                nsl = ds(nt * NW, NW)
                wb = w_tiles[nt]
                wb3 = wb[:, :].rearrange("p (kt n) -> p kt n", kt=KT)
                hsl = ds(half * HALF, HALF)
                nc.sync.dma_start(out=wb3[:, hsl, :], in_=w_r[:, hsl, nsl])
                for c in range(HALF // BIN_KT):
                    sl = ds((half * HALF + c * BIN_KT) * NW, BIN_KT * NW)
                    nc.vector.tensor_scalar(
                        wb[:, sl], wb[:, sl], 0.0, 0.5, ge, sub
                    )

            def alloc_w_tiles(nt):
                wb = wpool.tile([P, KT * NW], MMDT, tag="wb", name=f"wb{nt}")
                w_tiles[nt] = wb

            def mm(ps, mt, t, wb3, start, stop):
                if fp8:
                    nc.tensor.matmul(
                        ps[:, :],
                        lhsT=xb3[:, 2 * t : 2 * t + 2, ds(mt * P, P)],
                        rhs=wb3[:, 2 * t : 2 * t + 2, :],
                        start=start, stop=stop, perf_mode=perf_mode,
                    )
                else:
                    nc.tensor.matmul(
                        ps[:, :],
                        lhsT=xb[:, ds(t * n_per + mt * P, P)],
                        rhs=wb3[:, t, :],
                        start=start, stop=stop,
                    )

            def evict(ps, mt, nt):
                ot = opool.tile([P, NW], F32, tag="ot")
                nc.scalar.activation(ot[:, :], ps[:, :], Copy, 0.0, OUT_SCALE)
                nc.sync.dma_start(
                    out=out[ds(mt * P, P), ds(nt * NW, NW)], in_=ot[:, :]
                )

            w_tiles = {}
            NK = KT // 2 if fp8 else KT  # MM k-iterations per psum group

            # Startup interleave: first half of w chunk 0, then x, then the
            # rest of w chunk 0 — so the PE can start at the first x k-tiles
            # and never waits on the second w half.
            alloc_w_tiles(0)
            load_x_chunk(0)
            load_w_chunk(0, 0)
            for c in range(1, X_CH // 2):
                load_x_chunk(c)
            if N_HALVES > 1:
                load_w_chunk(0, 1)
            for c in range(X_CH // 2, X_CH):
                load_x_chunk(c)

            # n-chunk 0: kt-outer across all MT psum banks, pacing the PE
            # behind the streaming x DMA instead of stalling on full x.
            ps0 = [
                pspool.tile([P, NW], F32, tag="ps", name=f"ps0_{i}")
                for i in range(MT)
            ]
            wb3_0 = w_tiles[0][:, :].rearrange("p (kt n) -> p kt n", kt=KT)
            for t in range(NK):
                for mt in range(MT):
                    mm(ps0[mt], mt, t, wb3_0, start=(t == 0), stop=(t == NK - 1))
            for mt in range(MT):
                evict(ps0[mt], mt, 0)

            # n-chunks 1..: mt-outer (staggered psum eviction)
            for nt in range(1, NCH):
                alloc_w_tiles(nt)
                for h in range(N_HALVES):
                    load_w_chunk(nt, h)
                wb3 = w_tiles[nt][:, :].rearrange(
                    "p (kt n) -> p kt n", kt=KT
                )
                for mt in range(MT):
                    ps = pspool.tile([P, NW], F32, tag="ps")
                    for t in range(NK):
                        mm(ps, mt, t, wb3, start=(t == 0), stop=(t == NK - 1))
                    evict(ps, mt, nt)

    nc.compile()
    return nc


def _get_program():
    key = (N_PER, D_IN, D_OUT, MODE)
    if key not in _PROGRAM_CACHE:
        _PROGRAM_CACHE[key] = build_program()
    return _PROGRAM_CACHE[key]


def _encode_fp8(v):
    """Sign-exact fp8e4 re-encode of fp32 data for the device binarizer.

    ml_dtypes.float8_e4m3 matches TRN FP8_EXP4 (max 240, overflow saturates
    to +/-Inf, underflow to +/-0 -- sign always survives in the result).
    The only sign-ambiguous encodings are +/-0, which we patch to +/-1:
    +0 covers true zeros (reference maps them to +1) and underflowed
    positives; -0 covers underflowed negatives. After the patch the device
    binarize (v >= 0, or Sign) reproduces sign(original fp32) exactly for
    EVERY possible input value.
    """
    f8 = ml_dtypes.float8_e4m3
    v8 = np.clip(v, -240.0, 240.0).astype(f8)
    z = v8 == 0
    if z.any():
        v8 = np.where(z, np.where(np.signbit(v8), -1.0, 1.0).astype(f8), v8)
    return v8


def shard_inputs(x, weight):
    """Host-side sharding/layout: dtype re-encode + per-shard transpose."""
    if fp8_in(MODE):
        xe = _encode_fp8(x)
        we = _encode_fp8(weight)
    else:
        bf16 = ml_dtypes.bfloat16
        xe = x.astype(bf16)
        we = weight.astype(bf16)
    we = np.ascontiguousarray(we)
    shards = [
        np.ascontiguousarray(xe[i * N_PER : (i + 1) * N_PER].T)
        for i in range(N_CORES)
    ]
    return [{"xt": shards[i], "w": we} for i in range(N_CORES)]


def kernel(x, weight):
    from concourse.bass_utils import run_bass_kernel_spmd

    nc = _get_program()
    in_maps = shard_inputs(np.asarray(x), np.asarray(weight))
    res = run_bass_kernel_spmd(nc, in_maps, list(range(N_CORES)))
    return np.concatenate(
        [res.results[i]["out"] for i in range(N_CORES)], axis=0
    )
